# revision 1
# baseline (speedup 1.0000x reference)
"""Trainium2 Bass kernel for nn_Net_91122026151953.

Net (per batch row b):
  xe = x.transpose(0,3,1,2).reshape(B, 240, 180)            # [B,T,180]
  h_enc = lstm_cell_zero_state(xe, Wenc, b)                 # sigmoid/tanh gates, no recurrence
  enc   = softmax(h_enc, axis=-1)
  h_dec = lstm_cell_zero_state(enc, Wdec, b)
  out   = softmax((h_dec.reshape(B,T*180) @ W_out.T + b_out).reshape(B,4,10), -1)

Strategy: pure data-parallel over 8 cores (256 rows each). Row-major
("orientation A") pipeline with bf16 matmuls + intermediates, fp32 final
stage.  All transcendentals use only tanh/exp (sigmoid(x) =
0.5*(1+tanh(x/2)), halves folded into weights/activation scales) so a
single ACT table set is used (exp_and_others), avoiding ~2.7us table
switches.  The decoder bias is folded into Wdec columns (softmax rows sum
to 1).  E^T / h_dec^T for the chained matmuls are produced with DMA xbar
transposes (bf16, 128x128 tiles).
"""

import os
import numpy as np
import ml_dtypes

import concourse.bass as bass
import concourse.tile as tile
from concourse import bacc, mybir
from concourse import bass_utils

BF16 = ml_dtypes.bfloat16
FP32 = np.float32

H = 180          # hidden
T = 240          # timesteps
G3 = 540         # 3 used gates (i, g, o)
NCORES = 8
B_FULL = 2048
BL = B_FULL // NCORES   # rows per core = 256
NB = 32                 # batch rows per block
NBLK = BL // NB         # 8 blocks
LB = 8                  # batch rows per x-load DMA
CPB = NB * T // 128     # chunks (128 rows) per block = 60
MG = 4                  # chunks per macro-group (elementwise batch)
PG = 2                  # chunks per PSUM gates tile
MGB = CPB // MG         # macro-groups per block = 15
TPM = T // MGB          # final-matmul t-steps interleaved per macro-group = 16

AF = mybir.ActivationFunctionType
ALU = mybir.AluOpType
DT = mybir.dt

_PROGRAM = None
LAST_RESULTS = None


def _nsplits(tile_off):
    """Split [tile_off, tile_off+540) at 512-elem (psum bank) boundaries."""
    out = []
    lo = tile_off
    end = tile_off + G3
    while lo < end:
        hi = min(end, ((lo // 512) + 1) * 512)
        out.append((lo, hi - lo))
        lo = hi
    return out


def _build_program():
    nc = bacc.Bacc(None, name="lstm_net")

    xaug = nc.dram_tensor("xaug", [BL, 181, T], DT.bfloat16, kind="ExternalInput")
    wenc = nc.dram_tensor("wenc", [181, G3], DT.bfloat16, kind="ExternalInput")
    wdec = nc.dram_tensor("wdec", [180, G3], DT.bfloat16, kind="ExternalInput")
    w3a = nc.dram_tensor("w3a", [128, T * 40], DT.bfloat16, kind="ExternalInput")
    w3b = nc.dram_tensor("w3b", [52, T * 40], DT.bfloat16, kind="ExternalInput")
    bout = nc.dram_tensor("bout", [40, 1], DT.float32, kind="ExternalInput")
    ident = nc.dram_tensor("ident", [64, 64], DT.float32, kind="ExternalInput")
    out = nc.dram_tensor("out", [BL, 40], DT.float32, kind="ExternalOutput")

    with tile.TileContext(nc) as tc:
        with (
            tc.tile_pool(name="consts", bufs=1) as consts,
            tc.tile_pool(name="xa", bufs=2) as xa_pool,
            tc.tile_pool(name="work", bufs=3) as work,
            tc.tile_pool(name="et", bufs=MG + 2) as et_pool,
            tc.tile_pool(name="hd", bufs=2) as hd_pool,
            tc.tile_pool(name="mini", bufs=2) as mini,
            tc.tile_pool(name="psum", bufs=1, space="PSUM") as psum_pool,
        ):
            # ---- constants ----
            wenc1 = consts.tile([128, G3], DT.bfloat16, tag="wenc1")
            wenc2 = consts.tile([53, G3], DT.bfloat16, tag="wenc2")
            wdec1 = consts.tile([128, G3], DT.bfloat16, tag="wdec1")
            wdec2 = consts.tile([52, G3], DT.bfloat16, tag="wdec2")
            w3a_sb = consts.tile([128, T * 40], DT.bfloat16, tag="w3a")
            w3b_sb = consts.tile([52, T * 40], DT.bfloat16, tag="w3b")
            bout_sb = consts.tile([40, 1], DT.float32, tag="bout")
            ident_sb = consts.tile([64, 64], DT.float32, tag="ident")
            nc.sync.dma_start(out=wenc1[:], in_=wenc[0:128, :])
            nc.sync.dma_start(out=wenc2[:], in_=wenc[128:181, :])
            nc.sync.dma_start(out=wdec1[:], in_=wdec[0:128, :])
            nc.sync.dma_start(out=wdec2[:], in_=wdec[128:180, :])
            nc.sync.dma_start(out=w3a_sb[:], in_=w3a[:, :])
            nc.sync.dma_start(out=w3b_sb[:], in_=w3b[:, :])
            nc.sync.dma_start(out=bout_sb[:], in_=bout[:, :])
            nc.sync.dma_start(out=ident_sb[:], in_=ident[:, :])

            def final_mm_steps(lg, hda_p, hdb_p, t0, t1):
                for t in range(t0, t1):
                    nc.tensor.matmul(
                        lg[:], w3a_sb[:, t * 40:(t + 1) * 40], hda_p[:, :, t],
                        start=(t == 0), stop=False)
                    nc.tensor.matmul(
                        lg[:], w3b_sb[:, t * 40:(t + 1) * 40], hdb_p[0:52, :, t],
                        start=False, stop=(t == T - 1))

            def mini_softmax(lg, blk_prev):
                lgs = mini.tile([40, NB], DT.float32, tag="lgs")
                nc.scalar.copy(lgs[:], lg[:])
                nc.vector.tensor_scalar(
                    lgs[:], lgs[:], bout_sb[:, 0:1], None, ALU.add)
                pst = psum_pool.tile([NB, 40], DT.float32, tag="pst", bufs=1)
                nc.tensor.transpose(pst[:], lgs[:], ident_sb[0:40, 0:40])
                eo = mini.tile([NB, 40], DT.float32, tag="eo")
                nc.scalar.activation(eo[:], pst[:], AF.Exp)
                s4 = mini.tile([NB, 4], DT.float32, tag="s4")
                r4 = mini.tile([NB, 4], DT.float32, tag="r4")
                nc.vector.tensor_reduce(
                    s4[:], eo[:].rearrange("p (g k) -> p g k", k=10),
                    axis=mybir.AxisListType.X, op=ALU.add)
                nc.vector.reciprocal(r4[:], s4[:])
                ob = mini.tile([NB, 40], DT.float32, tag="ob")
                for g in range(4):
                    nc.vector.tensor_scalar(
                        ob[:, g * 10:(g + 1) * 10], eo[:, g * 10:(g + 1) * 10],
                        r4[:, g:g + 1], None, ALU.mult)
                nc.sync.dma_start(
                    out=out[blk_prev * NB:(blk_prev + 1) * NB, :], in_=ob[:])

            prev_hd = None  # (hda, hdb) of previous block
            lg_prev = None

            for blk in range(NBLK):
                # ---- x loads (feature-major slabs, LB batch rows each) ----
                xt1s, xt2s = [], []
                for l in range(NB // LB):
                    b0 = blk * NB + l * LB
                    xt1 = xa_pool.tile([128, LB, T], DT.bfloat16, tag="xt1")
                    xt2 = xa_pool.tile([53, LB, T], DT.bfloat16, tag="xt2")
                    nc.sync.dma_start(
                        out=xt1[:],
                        in_=xaug[b0:b0 + LB, 0:128, :].rearrange("b f t -> f b t"),
                    )
                    nc.sync.dma_start(
                        out=xt2[:],
                        in_=xaug[b0:b0 + LB, 128:181, :].rearrange("b f t -> f b t"),
                    )
                    xt1s.append(xt1)
                    xt2s.append(xt2)

                hda = hd_pool.tile([128, NB, T], DT.bfloat16, tag="hda")
                hdb = hd_pool.tile([128, NB, T], DT.bfloat16, tag="hdb")
                hda_f = hda[:].rearrange("p b t -> p (b t)")
                hdb_f = hdb[:].rearrange("p b t -> p (b t)")

                if prev_hd is not None:
                    lg_prev = psum_pool.tile([40, NB], DT.float32,
                                             tag="logits", bufs=1)

                chunks_per_load = (LB * T) // 128  # 15

                for mg in range(CPB // MG):
                    # interleave previous block's output-linear matmuls so the
                    # PE work spreads across this block instead of bursting
                    if prev_hd is not None:
                        final_mm_steps(lg_prev, prev_hd[0], prev_hd[1],
                                       mg * TPM, (mg + 1) * TPM)
                    tg_e = work.tile([128, MG, G3], DT.bfloat16, tag="tg_e")
                    w2c_e = work.tile([128, MG, H], DT.bfloat16, tag="w2c_e")
                    tc_e = work.tile([128, MG, H], DT.bfloat16, tag="tc_e")
                    h2_e = work.tile([128, MG, H], DT.bfloat16, tag="h2_e")
                    E6 = work.tile([128, MG, 256], DT.bfloat16, tag="E6")
                    s6 = work.tile([128, MG], DT.float32, tag="s6")
                    r6 = work.tile([128, MG], DT.float32, tag="r6")

                    # ---------- encoder matmuls + gate tanh ----------
                    for pgi in range(MG // PG):
                        ps = psum_pool.tile([128, PG * G3], DT.float32, tag="enc_gates", bufs=1)
                        for c in range(PG):
                            cc = mg * MG + pgi * PG + c
                            l, j = divmod(cc, chunks_per_load)
                            lhs1 = xt1s[l][:].rearrange("p b t -> p (b t)")[
                                :, j * 128:(j + 1) * 128]
                            lhs2 = xt2s[l][:].rearrange("p b t -> p (b t)")[
                                :, j * 128:(j + 1) * 128]
                            splits = _nsplits(c * G3)
                            for (n0, nw) in splits:
                                nc.tensor.matmul(
                                    ps[:, n0:n0 + nw], lhs1,
                                    wenc1[:, n0 - c * G3:n0 - c * G3 + nw],
                                    start=True, stop=False)
                            for (n0, nw) in splits:
                                nc.tensor.matmul(
                                    ps[:, n0:n0 + nw], lhs2,
                                    wenc2[:, n0 - c * G3:n0 - c * G3 + nw],
                                    start=False, stop=True)
                        nc.scalar.activation(
                            tg_e[:, pgi * PG:(pgi + 1) * PG, :],
                            ps[:].rearrange("p (c g) -> p c g", g=G3),
                            AF.Tanh)

                    # ---------- encoder cell elementwise ----------
                    i_sl = tg_e[:, :, 0:H]
                    g_sl = tg_e[:, :, H:2 * H]
                    o_sl = tg_e[:, :, 2 * H:3 * H]
                    # w2c = (tanh(i/2)+1)*tanh(g) = 2*c
                    nc.vector.scalar_tensor_tensor(
                        w2c_e[:], i_sl, 1.0, g_sl, ALU.add, ALU.mult)
                    nc.scalar.activation(tc_e[:], w2c_e[:], AF.Tanh, scale=0.5)
                    # h2 = (tanh(o/2)+1)*tanh(c) = 2*h
                    nc.vector.scalar_tensor_tensor(
                        h2_e[:], o_sl, 1.0, tc_e[:], ALU.add, ALU.mult)
                    nc.scalar.activation(E6[:, :, 0:H], h2_e[:], AF.Exp, scale=0.5)
                    nc.vector.tensor_reduce(
                        s6[:], E6[:, :, 0:H], axis=mybir.AxisListType.X, op=ALU.add)
                    nc.vector.reciprocal(r6[:], s6[:])

                    ets = []
                    for c in range(MG):
                        et1 = et_pool.tile([128, 128], DT.bfloat16, tag="et1")
                        et2 = et_pool.tile([128, 128], DT.bfloat16, tag="et2")
                        nc.sync.dma_start_transpose(et1[:], E6[:, c, 0:128])
                        nc.sync.dma_start_transpose(et2[:], E6[:, c, 128:256])
                        ets.append((et1, et2))

                    # ---------- decoder matmuls + gate tanh ----------
                    tg_d = work.tile([128, MG, G3], DT.bfloat16, tag="tg_d")
                    w2c_d = work.tile([128, MG, H], DT.bfloat16, tag="w2c_d")
                    tc_d = work.tile([128, MG, H], DT.bfloat16, tag="tc_d")
                    h2d6 = work.tile([128, MG, 256], DT.bfloat16, tag="h2d6")
                    for pgi in range(MG // PG):
                        psd = psum_pool.tile([128, PG * G3], DT.float32, tag="dec_gates", bufs=1)
                        for c in range(PG):
                            et1, et2 = ets[pgi * PG + c]
                            splits = _nsplits(c * G3)
                            for (n0, nw) in splits:
                                nc.tensor.matmul(
                                    psd[:, n0:n0 + nw], et1[:],
                                    wdec1[:, n0 - c * G3:n0 - c * G3 + nw],
                                    start=True, stop=False)
                            for (n0, nw) in splits:
                                nc.tensor.matmul(
                                    psd[:, n0:n0 + nw], et2[0:52, :],
                                    wdec2[:, n0 - c * G3:n0 - c * G3 + nw],
                                    start=False, stop=True)
                        for c in range(PG):
                            nc.scalar.activation(
                                tg_d[:, pgi * PG + c, :],
                                psd[:, c * G3:(c + 1) * G3],
                                AF.Tanh,
                                scale=r6[:, pgi * PG + c:pgi * PG + c + 1])

                    # ---------- decoder cell elementwise ----------
                    i_d = tg_d[:, :, 0:H]
                    g_d = tg_d[:, :, H:2 * H]
                    o_d = tg_d[:, :, 2 * H:3 * H]
                    nc.vector.scalar_tensor_tensor(
                        w2c_d[:], i_d, 1.0, g_d, ALU.add, ALU.mult)
                    nc.scalar.activation(tc_d[:], w2c_d[:], AF.Tanh, scale=0.5)
                    nc.vector.scalar_tensor_tensor(
                        h2d6[:, :, 0:H], o_d, 1.0, tc_d[:], ALU.add, ALU.mult)

                    for c in range(MG):
                        cc = mg * MG + c
                        nc.sync.dma_start_transpose(
                            hda_f[:, cc * 128:(cc + 1) * 128], h2d6[:, c, 0:128])
                        nc.sync.dma_start_transpose(
                            hdb_f[:, cc * 128:(cc + 1) * 128], h2d6[:, c, 128:256])

                # end of macro-group loop: previous block's logits are done
                if prev_hd is not None:
                    mini_softmax(lg_prev, blk - 1)
                prev_hd = (hda, hdb)

            # tail: last block's output linear + softmax
            lg_prev = psum_pool.tile([40, NB], DT.float32, tag="logits", bufs=1)
            final_mm_steps(lg_prev, prev_hd[0], prev_hd[1], 0, T)
            mini_softmax(lg_prev, NBLK - 1)

    nc.finalize()
    return nc


def _get_program():
    global _PROGRAM
    if _PROGRAM is None:
        _PROGRAM = _build_program()
    return _PROGRAM


def _prep_lstm_weights(Wih, bih, bhh):
    W = np.asarray(Wih, np.float32)
    b = np.asarray(bih, np.float32) + np.asarray(bhh, np.float32)
    # torch gate order i, f, g, o; f unused (zero state). Halve i/o for
    # the tanh half-angle sigmoid identity.
    Wp = np.concatenate([0.5 * W[0:H], W[2 * H:3 * H], 0.5 * W[3 * H:4 * H]], 0)
    bp = np.concatenate([0.5 * b[0:H], b[2 * H:3 * H], 0.5 * b[3 * H:4 * H]], 0)
    return Wp, bp  # [540, 180], [540]


def kernel(x, W_ih_enc, b_ih_enc, b_hh_enc, W_ih_dec, b_ih_dec, b_hh_dec,
           W_out, b_out):
    global LAST_RESULTS
    x = np.asarray(x)
    B = x.shape[0]
    assert B == B_FULL, f"kernel hardcoded for B={B_FULL}, got {B}"

    # x[b, c, s, t] with feature f = c*60+s -> xaug[b, f, t]; row of ones
    # provides the encoder bias via the augmented contraction dim.
    xaug = np.empty((B, 181, T), BF16)
    xaug[:, :180, :] = x.reshape(B, 180, T)
    xaug[:, 180, :] = 1.0

    We, be = _prep_lstm_weights(W_ih_enc, b_ih_enc, b_hh_enc)
    wenc = np.concatenate([We.T, be[None, :]], 0).astype(BF16)  # [181, 540]

    Wd, bd = _prep_lstm_weights(W_ih_dec, b_ih_dec, b_hh_dec)
    # softmax rows sum to 1 -> bias folds into every column of Wdec
    wdec = (Wd.T + bd[None, :]).astype(BF16)  # [180, 540]

    # logits use h = h2/2 -> fold the 0.5 into W_out; W3[h, t, j]
    W3 = (0.5 * np.asarray(W_out, np.float32)).reshape(40, T, H)
    W3 = np.ascontiguousarray(W3.transpose(2, 1, 0))  # [180, 240, 40]
    w3a = np.ascontiguousarray(W3[0:128]).reshape(128, T * 40).astype(BF16)
    w3b = np.ascontiguousarray(W3[128:180]).reshape(52, T * 40).astype(BF16)

    bout = np.asarray(b_out, np.float32).reshape(40, 1)
    ident = np.eye(64, dtype=np.float32)

    nc = _get_program()
    in_maps = []
    for c in range(NCORES):
        in_maps.append({
            "xaug": xaug[c * BL:(c + 1) * BL],
            "wenc": wenc,
            "wdec": wdec,
            "w3a": w3a,
            "w3b": w3b,
            "bout": bout,
            "ident": ident,
        })
    trace = bool(int(os.environ.get("KERNEL_TRACE", "0")))
    res = bass_utils.run_bass_kernel_spmd(
        nc, in_maps, core_ids=list(range(NCORES)), trace=trace)
    LAST_RESULTS = res
    out = np.concatenate([r["out"] for r in res.results], 0)  # [B, 40]
    return out.reshape(B, 4, 10).astype(np.float32)



# revision 3
# speedup vs baseline: 2.2738x; 2.2738x over previous
"""Trainium2 Bass kernel for nn_Net_91122026151953.

Net (per batch row b):
  xe = x.transpose(0,3,1,2).reshape(B, 240, 180)            # [B,T,180]
  h_enc = lstm_cell_zero_state(xe, Wenc, b)                 # sigmoid/tanh gates, no recurrence
  enc   = softmax(h_enc, axis=-1)
  h_dec = lstm_cell_zero_state(enc, Wdec, b)
  out   = softmax((h_dec.reshape(B,T*180) @ W_out.T + b_out).reshape(B,4,10), -1)

Strategy: pure data-parallel over 8 cores (256 batch rows each).
Row-major pipeline with t-major row ordering: the host pre-transposes x
per core to [181, T*256] with column index t*256+b (feature-major, so x
slabs load straight into the matmul stationary layout, and transposed
h_dec chunks are [h, 128 b] slices at a fixed t which feed N=128 moving
operands of the output linear, accumulated over all t into one
persistent PSUM tile [40, 256]).

All transcendentals use only tanh/exp (sigmoid(x) = 0.5*(1+tanh(x/2)),
halves folded into weights/activation scales) so a single ACT table set
is used.  The decoder bias is folded into Wdec columns (softmax rows sum
to 1) and the softmax normalizer 1/s is applied as the per-partition ACT
scale on the decoder gates.  E^T / h_dec^T are produced with batched DMA
xbar transposes ([128, 4*256] bf16 -> [128, 8, 128] in one call).
"""

import os
import numpy as np
import ml_dtypes

import concourse.bass as bass
import concourse.tile as tile
from concourse import bacc, mybir
from concourse import bass_utils

BF16 = ml_dtypes.bfloat16
FP32 = np.float32

H = 180          # hidden
T = 240          # timesteps
G3 = 540         # 3 used gates (i, g, o)
NCORES = 8
B_FULL = 2048
BL = B_FULL // NCORES   # batch rows per core = 256
MG = 4                  # chunks (128 rows) per macro-group
RPM = MG * 128          # rows per macro-group = 512
NMG = BL * T // RPM     # macro-groups per core = 120
XPF = 2                 # x-slab prefetch distance (macro-groups)

AF = mybir.ActivationFunctionType
ALU = mybir.AluOpType
DT = mybir.dt

_PROGRAM = None
LAST_RESULTS = None


def _nsplits(tile_off, width=G3):
    """Split [tile_off, tile_off+width) at 512-elem (psum bank) boundaries."""
    out = []
    lo = tile_off
    end = tile_off + width
    while lo < end:
        hi = min(end, ((lo // 512) + 1) * 512)
        out.append((lo, hi - lo))
        lo = hi
    return out


def _build_program():
    nc = bacc.Bacc(None, name="lstm_net2")

    xaug = nc.dram_tensor("xaug", [181, T * BL], DT.bfloat16, kind="ExternalInput")
    wenc = nc.dram_tensor("wenc", [181, G3], DT.bfloat16, kind="ExternalInput")
    wdec = nc.dram_tensor("wdec", [180, G3], DT.bfloat16, kind="ExternalInput")
    w3a = nc.dram_tensor("w3a", [128, T * 40], DT.bfloat16, kind="ExternalInput")
    w3b = nc.dram_tensor("w3b", [52, T * 40], DT.bfloat16, kind="ExternalInput")
    bout = nc.dram_tensor("bout", [40, 1], DT.float32, kind="ExternalInput")
    ident = nc.dram_tensor("ident", [64, 64], DT.float32, kind="ExternalInput")
    out = nc.dram_tensor("out", [BL, 40], DT.float32, kind="ExternalOutput")

    with tile.TileContext(nc) as tc:
        with (
            tc.tile_pool(name="consts", bufs=1) as consts,
            tc.tile_pool(name="xa", bufs=XPF + 2) as xa_pool,
            tc.tile_pool(name="work", bufs=2) as work,
            tc.tile_pool(name="et", bufs=2) as et_pool,
            tc.tile_pool(name="hd", bufs=2) as hd_pool,
            tc.tile_pool(name="mini", bufs=1) as mini,
            tc.tile_pool(name="psum", bufs=1, space="PSUM") as psum_pool,
        ):
            # ---- constants ----
            wenc1 = consts.tile([128, G3], DT.bfloat16, tag="wenc1")
            wenc2 = consts.tile([53, G3], DT.bfloat16, tag="wenc2")
            wdec1 = consts.tile([128, G3], DT.bfloat16, tag="wdec1")
            wdec2 = consts.tile([52, G3], DT.bfloat16, tag="wdec2")
            w3a_sb = consts.tile([128, T * 40], DT.bfloat16, tag="w3a")
            w3b_sb = consts.tile([52, T * 40], DT.bfloat16, tag="w3b")
            bout_sb = consts.tile([40, 1], DT.float32, tag="bout")
            ident_sb = consts.tile([64, 64], DT.float32, tag="ident")
            nc.sync.dma_start(out=wenc1[:], in_=wenc[0:128, :])
            nc.sync.dma_start(out=wenc2[:], in_=wenc[128:181, :])
            nc.sync.dma_start(out=wdec1[:], in_=wdec[0:128, :])
            nc.sync.dma_start(out=wdec2[:], in_=wdec[128:180, :])
            nc.sync.dma_start(out=w3a_sb[:], in_=w3a[:, :])
            nc.sync.dma_start(out=w3b_sb[:], in_=w3b[:, :])
            nc.sync.dma_start(out=bout_sb[:], in_=bout[:, :])
            nc.sync.dma_start(out=ident_sb[:], in_=ident[:, :])

            # persistent output-linear accumulator [40 logits, 256 b]
            lg = psum_pool.tile([40, BL], DT.float32, tag="logits", bufs=1)

            def load_x(mg):
                c0 = mg * RPM
                xt1 = xa_pool.tile([128, RPM], DT.bfloat16, tag="xt1")
                xt2 = xa_pool.tile([53, RPM], DT.bfloat16, tag="xt2")
                nc.sync.dma_start(out=xt1[:], in_=xaug[0:128, c0:c0 + RPM])
                nc.sync.dma_start(out=xt2[:], in_=xaug[128:181, c0:c0 + RPM])
                return xt1, xt2

            def encoder_stage(mg, xt):
                """enc matmuls + cell elementwise + softmax exp; emits E^T."""
                xt1, xt2 = xt
                tg_e = work.tile([128, MG, G3], DT.bfloat16, tag="tg_e")
                w2c_e = work.tile([128, MG, H], DT.bfloat16, tag="w2c_e")
                tc_e = work.tile([128, MG, H], DT.bfloat16, tag="tc_e")
                h2_e = work.tile([128, MG, H], DT.bfloat16, tag="h2_e")
                E6 = work.tile([128, MG, 256], DT.bfloat16, tag="E6")
                s6 = work.tile([128, MG], DT.float32, tag="s6")
                r6 = work.tile([128, MG], DT.float32, tag="r6")

                for pg in range(MG // 2):
                    ps = psum_pool.tile([128, 2 * G3], DT.float32,
                                        tag="enc_gates", bufs=1)
                    for c in range(2):
                        cc = pg * 2 + c
                        lhs1 = xt1[:, cc * 128:(cc + 1) * 128]
                        lhs2 = xt2[:, cc * 128:(cc + 1) * 128]
                        splits = _nsplits(c * G3)
                        for (n0, nw) in splits:
                            nc.tensor.matmul(
                                ps[:, n0:n0 + nw], lhs1,
                                wenc1[:, n0 - c * G3:n0 - c * G3 + nw],
                                start=True, stop=False)
                        for (n0, nw) in splits:
                            nc.tensor.matmul(
                                ps[:, n0:n0 + nw], lhs2,
                                wenc2[:, n0 - c * G3:n0 - c * G3 + nw],
                                start=False, stop=True)
                    nc.scalar.activation(
                        tg_e[:, pg * 2:(pg + 1) * 2, :],
                        ps[:].rearrange("p (c g) -> p c g", g=G3),
                        AF.Tanh)

                i_sl = tg_e[:, :, 0:H]
                g_sl = tg_e[:, :, H:2 * H]
                o_sl = tg_e[:, :, 2 * H:3 * H]
                # w2c = (tanh(i/2)+1)*tanh(g) = 2*c
                nc.vector.scalar_tensor_tensor(
                    w2c_e[:], i_sl, 1.0, g_sl, ALU.add, ALU.mult)
                nc.scalar.activation(tc_e[:], w2c_e[:], AF.Tanh, scale=0.5)
                # h2 = (tanh(o/2)+1)*tanh(c) = 2*h
                nc.vector.scalar_tensor_tensor(
                    h2_e[:], o_sl, 1.0, tc_e[:], ALU.add, ALU.mult)
                nc.scalar.activation(E6[:, :, 0:H], h2_e[:], AF.Exp, scale=0.5)
                nc.vector.tensor_reduce(
                    s6[:], E6[:, :, 0:H], axis=mybir.AxisListType.X, op=ALU.add)
                nc.vector.reciprocal(r6[:], s6[:])

                # batched xbar transpose: E6 [128, MG*256] -> [128, 2*MG, 128]
                # ET[p, 2c+half, i] = E6[i, c, 128*half + p]
                ET = et_pool.tile([128, 2 * MG, 128], DT.bfloat16, tag="ET")
                nc.sync.dma_start_transpose(
                    ET[:], E6[:].rearrange("p c f -> p (c f)"))
                return ET, r6

            def decoder_stage(mg, enc_out):
                """dec matmuls (stationary E^T chunks) + cell; emits h_dec^T."""
                ET, r6 = enc_out
                tg_d = work.tile([128, MG, G3], DT.bfloat16, tag="tg_d")
                w2c_d = work.tile([128, MG, H], DT.bfloat16, tag="w2c_d")
                tc_d = work.tile([128, MG, H], DT.bfloat16, tag="tc_d")
                h2d6 = work.tile([128, MG, 256], DT.bfloat16, tag="h2d6")

                for c in range(MG):
                    psd = psum_pool.tile([128, G3], DT.float32,
                                         tag="dec_gates", bufs=2)
                    et1 = ET[:, 2 * c, :]
                    et2 = ET[0:52, 2 * c + 1, :]
                    for (n0, nw) in _nsplits(0):
                        nc.tensor.matmul(
                            psd[:, n0:n0 + nw], et1, wdec1[:, n0:n0 + nw],
                            start=True, stop=False)
                    for (n0, nw) in _nsplits(0):
                        nc.tensor.matmul(
                            psd[:, n0:n0 + nw], et2, wdec2[:, n0:n0 + nw],
                            start=False, stop=True)
                    nc.scalar.activation(
                        tg_d[:, c, :], psd[:], AF.Tanh,
                        scale=r6[:, c:c + 1])

                i_d = tg_d[:, :, 0:H]
                g_d = tg_d[:, :, H:2 * H]
                o_d = tg_d[:, :, 2 * H:3 * H]
                nc.vector.scalar_tensor_tensor(
                    w2c_d[:], i_d, 1.0, g_d, ALU.add, ALU.mult)
                nc.scalar.activation(tc_d[:], w2c_d[:], AF.Tanh, scale=0.5)
                nc.vector.scalar_tensor_tensor(
                    h2d6[:, :, 0:H], o_d, 1.0, tc_d[:], ALU.add, ALU.mult)

                HDT = hd_pool.tile([128, 2 * MG, 128], DT.bfloat16, tag="HDT")
                nc.sync.dma_start_transpose(
                    HDT[:], h2d6[:].rearrange("p c f -> p (c f)"))
                return HDT

            def final_stage(mg, HDT):
                """output linear: accumulate 2*MG j-slices into lg."""
                for c in range(MG):
                    gc = mg * MG + c        # global 128-row chunk
                    t = gc // 2
                    bh = gc % 2             # which 128-batch half
                    osl = lg[:, bh * 128:(bh + 1) * 128]
                    nc.tensor.matmul(
                        osl, w3a_sb[:, t * 40:(t + 1) * 40],
                        HDT[:, 2 * c, :],
                        start=(t == 0), stop=False)
                    nc.tensor.matmul(
                        osl, w3b_sb[:, t * 40:(t + 1) * 40],
                        HDT[0:52, 2 * c + 1, :],
                        start=False, stop=(t == T - 1))

            def out_softmax():
                lgs = mini.tile([40, BL], DT.float32, tag="lgs")
                nc.vector.tensor_scalar(
                    lgs[:], lg[:], bout_sb[:, 0:1], None, ALU.add)
                eo = mini.tile([128, 2, 40], DT.float32, tag="eo")
                for half in range(2):
                    # reuse the dec_gates psum allocation (kernel is done
                    # with decoding by now); only cols 0:40 are written
                    pst = psum_pool.tile([128, G3], DT.float32,
                                         tag="dec_gates", bufs=2)
                    nc.tensor.transpose(
                        pst[:, 0:40], lgs[:, half * 128:(half + 1) * 128],
                        ident_sb[0:40, 0:40])
                    nc.scalar.activation(eo[:, half, :], pst[:, 0:40], AF.Exp)
                s4 = mini.tile([128, 2, 4], DT.float32, tag="s4")
                r4 = mini.tile([128, 2, 4], DT.float32, tag="r4")
                nc.vector.tensor_reduce(
                    s4[:], eo[:].rearrange("p h (g k) -> p h g k", k=10),
                    axis=mybir.AxisListType.X, op=ALU.add)
                nc.vector.reciprocal(r4[:], s4[:])
                ob = mini.tile([128, 2, 40], DT.float32, tag="ob")
                for half in range(2):
                    for g in range(4):
                        nc.vector.tensor_scalar(
                            ob[:, half, g * 10:(g + 1) * 10],
                            eo[:, half, g * 10:(g + 1) * 10],
                            r4[:, half, g:g + 1], None, ALU.mult)
                for half in range(2):
                    nc.sync.dma_start(
                        out=out[half * 128:(half + 1) * 128, :],
                        in_=ob[:, half, :])

            # ---- software pipeline: enc(mg) | dec(mg-1) | final(mg-2) ----
            xt_q = {}
            enc_q = {}
            hdt_q = {}
            for mg in range(min(XPF, NMG)):
                xt_q[mg] = load_x(mg)
            for it in range(NMG + 2):
                mg_f = it - 2
                if 0 <= mg_f < NMG:
                    final_stage(mg_f, hdt_q.pop(mg_f))
                if it < NMG:
                    if it + XPF < NMG:
                        xt_q[it + XPF] = load_x(it + XPF)
                    enc_q[it] = encoder_stage(it, xt_q.pop(it))
                mg_d = it - 1
                if 0 <= mg_d < NMG:
                    hdt_q[mg_d] = decoder_stage(mg_d, enc_q.pop(mg_d))

            out_softmax()

    nc.finalize()
    return nc


def _get_program():
    global _PROGRAM
    if _PROGRAM is None:
        _PROGRAM = _build_program()
    return _PROGRAM


def _prep_lstm_weights(Wih, bih, bhh):
    W = np.asarray(Wih, np.float32)
    b = np.asarray(bih, np.float32) + np.asarray(bhh, np.float32)
    # torch gate order i, f, g, o; f unused (zero state). Halve i/o for
    # the tanh half-angle sigmoid identity.
    Wp = np.concatenate([0.5 * W[0:H], W[2 * H:3 * H], 0.5 * W[3 * H:4 * H]], 0)
    bp = np.concatenate([0.5 * b[0:H], b[2 * H:3 * H], 0.5 * b[3 * H:4 * H]], 0)
    return Wp, bp  # [540, 180], [540]


def kernel(x, W_ih_enc, b_ih_enc, b_hh_enc, W_ih_dec, b_ih_dec, b_hh_dec,
           W_out, b_out):
    global LAST_RESULTS
    x = np.asarray(x)
    B = x.shape[0]
    assert B == B_FULL, f"kernel hardcoded for B={B_FULL}, got {B}"

    # x[b, c, s, t] with feature f = c*60+s; per core transpose to
    # [f, t, b] (column index t*BL + b) with a trailing ones row
    # providing the encoder bias via the augmented contraction dim.
    xf = x.reshape(B, H, T).astype(BF16)

    We, be = _prep_lstm_weights(W_ih_enc, b_ih_enc, b_hh_enc)
    wenc = np.concatenate([We.T, be[None, :]], 0).astype(BF16)  # [181, 540]

    Wd, bd = _prep_lstm_weights(W_ih_dec, b_ih_dec, b_hh_dec)
    # softmax rows sum to 1 -> bias folds into every column of Wdec
    wdec = (Wd.T + bd[None, :]).astype(BF16)  # [180, 540]

    # logits use h = h2/2 -> fold the 0.5 into W_out; W3[h, t, j]
    W3 = (0.5 * np.asarray(W_out, np.float32)).reshape(40, T, H)
    W3 = np.ascontiguousarray(W3.transpose(2, 1, 0))  # [180, 240, 40]
    w3a = np.ascontiguousarray(W3[0:128]).reshape(128, T * 40).astype(BF16)
    w3b = np.ascontiguousarray(W3[128:180]).reshape(52, T * 40).astype(BF16)

    bout = np.asarray(b_out, np.float32).reshape(40, 1)
    ident = np.eye(64, dtype=np.float32)

    nc = _get_program()
    in_maps = []
    for c in range(NCORES):
        xc = xf[c * BL:(c + 1) * BL]                       # [256, 180, 240]
        xtc = np.ascontiguousarray(xc.transpose(1, 2, 0))  # [180, 240, 256]
        xaug = np.empty((181, T * BL), BF16)
        xaug[:180] = xtc.reshape(H, T * BL)
        xaug[180] = 1.0
        in_maps.append({
            "xaug": xaug,
            "wenc": wenc,
            "wdec": wdec,
            "w3a": w3a,
            "w3b": w3b,
            "bout": bout,
            "ident": ident,
        })
    trace = bool(int(os.environ.get("KERNEL_TRACE", "0")))
    res = bass_utils.run_bass_kernel_spmd(
        nc, in_maps, core_ids=list(range(NCORES)), trace=trace)
    LAST_RESULTS = res
    out = np.concatenate([r["out"] for r in res.results], 0)  # [B, 40]
    return out.reshape(B, 4, 10).astype(np.float32)


# revision 8
# speedup vs baseline: 2.3928x; 1.0523x over previous
"""Trainium2 Bass kernel for nn_Net_91122026151953.

Net (per batch row b):
  xe = x.transpose(0,3,1,2).reshape(B, 240, 180)            # [B,T,180]
  h_enc = lstm_cell_zero_state(xe, Wenc, b)                 # sigmoid/tanh gates, no recurrence
  enc   = softmax(h_enc, axis=-1)
  h_dec = lstm_cell_zero_state(enc, Wdec, b)
  out   = softmax((h_dec.reshape(B,T*180) @ W_out.T + b_out).reshape(B,4,10), -1)

Strategy: pure data-parallel over 8 cores (256 batch rows each).
Row-major pipeline with t-major row ordering: the host pre-transposes x
per core to [181, T*256] with column index t*256+b (feature-major, so x
slabs load straight into the matmul stationary layout, and transposed
h_dec chunks are [h, 128 b] slices at a fixed t which feed N=128 moving
operands of the output linear, accumulated over all t into one
persistent PSUM tile [40, 256]).

All transcendentals use only tanh/exp (sigmoid(x) = 0.5*(1+tanh(x/2)),
halves folded into weights/activation scales) so a single ACT table set
is used.  The decoder bias is folded into Wdec columns (softmax rows sum
to 1) and the softmax normalizer 1/s is applied as the per-partition ACT
scale on the decoder gates.  E^T / h_dec^T are produced with batched DMA
xbar transposes ([128, 4*256] bf16 -> [128, 8, 128] in one call).
"""

import os
import numpy as np
import ml_dtypes

import concourse.bass as bass
import concourse.tile as tile
from concourse import bacc, mybir
from concourse import bass_utils

BF16 = ml_dtypes.bfloat16
FP32 = np.float32

H = 180          # hidden
T = 240          # timesteps
G3 = 540         # 3 used gates (i, g, o)
NCORES = 8
B_FULL = 2048
BL = B_FULL // NCORES   # batch rows per core = 256
MG = 4                  # chunks (128 rows) per macro-group
RPM = MG * 128          # rows per macro-group = 512
NMG = BL * T // RPM     # macro-groups per core = 120
XPF = 2                 # x-slab prefetch distance (macro-groups)

AF = mybir.ActivationFunctionType
ALU = mybir.AluOpType
DT = mybir.dt

_PROGRAM = None
LAST_RESULTS = None


def _ldw_key(inst):
    """Identity key for an InstLdweights / matmul-weights AP."""
    ap = inst.ins[-1]
    bap = ap.bass_ap
    return (bap.tensor.name, bap.offset, tuple(map(tuple, bap.ap)),
            getattr(inst, 'tile_position', None),
            getattr(inst, 'perf_mode', None),
            getattr(inst, 'is_transpose', None))


def _dedup_ldweights(nc):
    """Remove back-to-back duplicate InstLdweights on the PE queue.

    The Tile scheduler emits one LDWEIGHTS per matmul; the 512/28
    psum-bank splits of a 540-wide moving operand reuse the same
    stationary, so every second load is redundant (~107ns of PE array
    time each).  Safe when the duplicate carries no sync_info and only
    InstMatmult with the same weights sits between the two loads.
    """
    removed = 0
    for blk in nc.m.functions[0].blocks:
        last_key = None
        to_remove = []
        for inst in blk.instructions:
            if getattr(inst, 'engine', None) != mybir.EngineType.PE:
                continue
            tn = type(inst).__name__
            if tn == 'InstLdweights':
                key = _ldw_key(inst)
                si = inst.sync_info
                clean = si is None or (not si.on_wait and not si.on_update)
                if key == last_key and clean:
                    to_remove.append(inst)
                else:
                    last_key = key
            elif tn == 'InstMatmult':
                if _ldw_key(inst) != last_key:
                    last_key = None
            else:
                last_key = None
        for inst in to_remove:
            blk.instructions.remove(inst)
            removed += 1
    return removed


def _nsplits(tile_off, width=G3):
    """Split [tile_off, tile_off+width) at 512-elem (psum bank) boundaries."""
    out = []
    lo = tile_off
    end = tile_off + width
    while lo < end:
        hi = min(end, ((lo // 512) + 1) * 512)
        out.append((lo, hi - lo))
        lo = hi
    return out


def _build_program():
    nc = bacc.Bacc(None, name="lstm_net2")

    xaug = nc.dram_tensor("xaug", [181, T * BL], DT.bfloat16, kind="ExternalInput")
    wenc = nc.dram_tensor("wenc", [181, G3], DT.bfloat16, kind="ExternalInput")
    wdec = nc.dram_tensor("wdec", [180, G3], DT.bfloat16, kind="ExternalInput")
    w3a = nc.dram_tensor("w3a", [128, T * 40], DT.bfloat16, kind="ExternalInput")
    w3b = nc.dram_tensor("w3b", [52, T * 40], DT.bfloat16, kind="ExternalInput")
    bout = nc.dram_tensor("bout", [40, 1], DT.float32, kind="ExternalInput")
    ident = nc.dram_tensor("ident", [64, 64], DT.float32, kind="ExternalInput")
    out = nc.dram_tensor("out", [BL, 40], DT.float32, kind="ExternalOutput")

    with tile.TileContext(nc) as tc:
        with (
            tc.tile_pool(name="consts", bufs=1) as consts,
            tc.tile_pool(name="xa", bufs=XPF + 2) as xa_pool,
            tc.tile_pool(name="work", bufs=3) as work,
            tc.tile_pool(name="et", bufs=3) as et_pool,
            tc.tile_pool(name="hd", bufs=3) as hd_pool,
            tc.tile_pool(name="mini", bufs=1) as mini,
            tc.tile_pool(name="psum", bufs=1, space="PSUM") as psum_pool,
        ):
            # ---- constants ----
            wenc1 = consts.tile([128, G3], DT.bfloat16, tag="wenc1")
            wenc2 = consts.tile([53, G3], DT.bfloat16, tag="wenc2")
            wdec1 = consts.tile([128, G3], DT.bfloat16, tag="wdec1")
            wdec2 = consts.tile([52, G3], DT.bfloat16, tag="wdec2")
            w3a_sb = consts.tile([128, T * 40], DT.bfloat16, tag="w3a")
            w3b_sb = consts.tile([52, T * 40], DT.bfloat16, tag="w3b")
            bout_sb = consts.tile([40, 1], DT.float32, tag="bout")
            ident_sb = consts.tile([64, 64], DT.float32, tag="ident")
            nc.sync.dma_start(out=wenc1[:], in_=wenc[0:128, :])
            nc.sync.dma_start(out=wenc2[:], in_=wenc[128:181, :])
            nc.sync.dma_start(out=wdec1[:], in_=wdec[0:128, :])
            nc.sync.dma_start(out=wdec2[:], in_=wdec[128:180, :])
            nc.sync.dma_start(out=w3a_sb[:], in_=w3a[:, :])
            nc.sync.dma_start(out=w3b_sb[:], in_=w3b[:, :])
            nc.sync.dma_start(out=bout_sb[:], in_=bout[:, :])
            nc.sync.dma_start(out=ident_sb[:], in_=ident[:, :])

            # persistent output-linear accumulator [40 logits, 256 b]
            lg = psum_pool.tile([40, BL], DT.float32, tag="logits", bufs=1)

            def load_x(mg):
                c0 = mg * RPM
                xt1 = xa_pool.tile([128, RPM], DT.bfloat16, tag="xt1")
                xt2 = xa_pool.tile([53, RPM], DT.bfloat16, tag="xt2")
                nc.sync.dma_start(out=xt1[:], in_=xaug[0:128, c0:c0 + RPM])
                nc.sync.dma_start(out=xt2[:], in_=xaug[128:181, c0:c0 + RPM])
                return xt1, xt2

            def encoder_stage(mg, xt):
                """enc matmuls + cell elementwise + softmax exp; emits E^T."""
                xt1, xt2 = xt
                tg_e = work.tile([128, MG, G3], DT.bfloat16, tag="tg_e")
                w2c_e = work.tile([128, MG, H], DT.bfloat16, tag="w2c_e")
                tc_e = work.tile([128, MG, H], DT.bfloat16, tag="tc_e")
                h2_e = work.tile([128, MG, H], DT.bfloat16, tag="h2_e")
                E6 = work.tile([128, MG, 256], DT.bfloat16, tag="E6")
                s6 = work.tile([128, MG], DT.float32, tag="s6")
                r6 = work.tile([128, MG], DT.float32, tag="r6")

                for pg in range(MG // 2):
                    ps = psum_pool.tile([128, 2 * G3], DT.float32,
                                        tag="enc_gates", bufs=1)
                    for c in range(2):
                        cc = pg * 2 + c
                        lhs1 = xt1[:, cc * 128:(cc + 1) * 128]
                        lhs2 = xt2[:, cc * 128:(cc + 1) * 128]
                        splits = _nsplits(c * G3)
                        for (n0, nw) in splits:
                            nc.tensor.matmul(
                                ps[:, n0:n0 + nw], lhs1,
                                wenc1[:, n0 - c * G3:n0 - c * G3 + nw],
                                start=True, stop=False)
                        for (n0, nw) in splits:
                            nc.tensor.matmul(
                                ps[:, n0:n0 + nw], lhs2,
                                wenc2[:, n0 - c * G3:n0 - c * G3 + nw],
                                start=False, stop=True)
                    nc.scalar.activation(
                        tg_e[:, pg * 2:(pg + 1) * 2, :],
                        ps[:].rearrange("p (c g) -> p c g", g=G3),
                        AF.Tanh)

                i_sl = tg_e[:, :, 0:H]
                g_sl = tg_e[:, :, H:2 * H]
                o_sl = tg_e[:, :, 2 * H:3 * H]
                # w2c = (tanh(i/2)+1)*tanh(g) = 2*c
                nc.vector.scalar_tensor_tensor(
                    w2c_e[:], i_sl, 1.0, g_sl, ALU.add, ALU.mult)
                nc.scalar.activation(tc_e[:], w2c_e[:], AF.Tanh, scale=0.5)
                # h2 = (tanh(o/2)+1)*tanh(c) = 2*h
                nc.vector.scalar_tensor_tensor(
                    h2_e[:], o_sl, 1.0, tc_e[:], ALU.add, ALU.mult)
                nc.scalar.activation(E6[:, :, 0:H], h2_e[:], AF.Exp, scale=0.5)
                nc.vector.tensor_reduce(
                    s6[:], E6[:, :, 0:H], axis=mybir.AxisListType.X, op=ALU.add)
                nc.vector.reciprocal(r6[:], s6[:])

                # batched xbar transpose: E6 [128, MG*256] -> [128, 2*MG, 128]
                # ET[p, 2c+half, i] = E6[i, c, 128*half + p]
                ET = et_pool.tile([128, 2 * MG, 128], DT.bfloat16, tag="ET")
                nc.sync.dma_start_transpose(
                    ET[:], E6[:].rearrange("p c f -> p (c f)"))
                return ET, r6

            def decoder_stage(mg, enc_out):
                """dec matmuls (stationary E^T chunks) + cell; emits h_dec^T."""
                ET, r6 = enc_out
                tg_d = work.tile([128, MG, G3], DT.bfloat16, tag="tg_d")
                w2c_d = work.tile([128, MG, H], DT.bfloat16, tag="w2c_d")
                tc_d = work.tile([128, MG, H], DT.bfloat16, tag="tc_d")
                h2d6 = work.tile([128, MG, 256], DT.bfloat16, tag="h2d6")

                for c in range(MG):
                    psd = psum_pool.tile([128, G3], DT.float32,
                                         tag="dec_gates", bufs=2)
                    et1 = ET[:, 2 * c, :]
                    et2 = ET[0:52, 2 * c + 1, :]
                    for (n0, nw) in _nsplits(0):
                        nc.tensor.matmul(
                            psd[:, n0:n0 + nw], et1, wdec1[:, n0:n0 + nw],
                            start=True, stop=False)
                    for (n0, nw) in _nsplits(0):
                        nc.tensor.matmul(
                            psd[:, n0:n0 + nw], et2, wdec2[:, n0:n0 + nw],
                            start=False, stop=True)
                    nc.scalar.activation(
                        tg_d[:, c, :], psd[:], AF.Tanh,
                        scale=r6[:, c:c + 1])

                i_d = tg_d[:, :, 0:H]
                g_d = tg_d[:, :, H:2 * H]
                o_d = tg_d[:, :, 2 * H:3 * H]
                nc.vector.scalar_tensor_tensor(
                    w2c_d[:], i_d, 1.0, g_d, ALU.add, ALU.mult)
                nc.scalar.activation(tc_d[:], w2c_d[:], AF.Tanh, scale=0.5)
                nc.vector.scalar_tensor_tensor(
                    h2d6[:, :, 0:H], o_d, 1.0, tc_d[:], ALU.add, ALU.mult)

                HDT = hd_pool.tile([128, 2 * MG, 128], DT.bfloat16, tag="HDT")
                nc.sync.dma_start_transpose(
                    HDT[:], h2d6[:].rearrange("p c f -> p (c f)"))
                return HDT

            def final_stage(mg, HDT):
                """output linear: accumulate into lg, N=256 moving (both
                128-batch halves of a timestep via a strided AP)."""
                for ct in range(MG // 2):
                    t = mg * (MG // 2) + ct
                    j0 = 4 * ct
                    nc.tensor.matmul(
                        lg[:], w3a_sb[:, t * 40:(t + 1) * 40],
                        HDT[:][:, j0:j0 + 4:2, :],
                        start=(t == 0), stop=False)
                    nc.tensor.matmul(
                        lg[:], w3b_sb[:, t * 40:(t + 1) * 40],
                        HDT[:][0:52, j0 + 1:j0 + 4:2, :],
                        start=False, stop=(t == T - 1))

            def out_softmax():
                lgs = mini.tile([40, BL], DT.float32, tag="lgs")
                nc.vector.tensor_scalar(
                    lgs[:], lg[:], bout_sb[:, 0:1], None, ALU.add)
                eo = mini.tile([128, 2, 40], DT.float32, tag="eo")
                for half in range(2):
                    # reuse the dec_gates psum allocation (kernel is done
                    # with decoding by now); only cols 0:40 are written
                    pst = psum_pool.tile([128, G3], DT.float32,
                                         tag="dec_gates", bufs=2)
                    nc.tensor.transpose(
                        pst[:, 0:40], lgs[:, half * 128:(half + 1) * 128],
                        ident_sb[0:40, 0:40])
                    nc.scalar.activation(eo[:, half, :], pst[:, 0:40], AF.Exp)
                s4 = mini.tile([128, 2, 4], DT.float32, tag="s4")
                r4 = mini.tile([128, 2, 4], DT.float32, tag="r4")
                nc.vector.tensor_reduce(
                    s4[:], eo[:].rearrange("p h (g k) -> p h g k", k=10),
                    axis=mybir.AxisListType.X, op=ALU.add)
                nc.vector.reciprocal(r4[:], s4[:])
                ob = mini.tile([128, 2, 40], DT.float32, tag="ob")
                for half in range(2):
                    for g in range(4):
                        nc.vector.tensor_scalar(
                            ob[:, half, g * 10:(g + 1) * 10],
                            eo[:, half, g * 10:(g + 1) * 10],
                            r4[:, half, g:g + 1], None, ALU.mult)
                for half in range(2):
                    nc.sync.dma_start(
                        out=out[half * 128:(half + 1) * 128, :],
                        in_=ob[:, half, :])

            # ---- software pipeline: enc(mg) | dec(mg-1) | final(mg-2) ----
            xt_q = {}
            enc_q = {}
            hdt_q = {}
            for mg in range(min(XPF, NMG)):
                xt_q[mg] = load_x(mg)
            for it in range(NMG + 2):
                mg_f = it - 2
                if 0 <= mg_f < NMG:
                    final_stage(mg_f, hdt_q.pop(mg_f))
                if it < NMG:
                    if it + XPF < NMG:
                        xt_q[it + XPF] = load_x(it + XPF)
                    enc_q[it] = encoder_stage(it, xt_q.pop(it))
                mg_d = it - 1
                if 0 <= mg_d < NMG:
                    hdt_q[mg_d] = decoder_stage(mg_d, enc_q.pop(mg_d))

            out_softmax()

    n = _dedup_ldweights(nc)
    assert n >= NMG * 2, f"ldweights dedup only removed {n}"
    nc.finalize()
    return nc


def _get_program():
    global _PROGRAM
    if _PROGRAM is None:
        _PROGRAM = _build_program()
    return _PROGRAM


def _prep_lstm_weights(Wih, bih, bhh):
    W = np.asarray(Wih, np.float32)
    b = np.asarray(bih, np.float32) + np.asarray(bhh, np.float32)
    # torch gate order i, f, g, o; f unused (zero state). Halve i/o for
    # the tanh half-angle sigmoid identity.
    Wp = np.concatenate([0.5 * W[0:H], W[2 * H:3 * H], 0.5 * W[3 * H:4 * H]], 0)
    bp = np.concatenate([0.5 * b[0:H], b[2 * H:3 * H], 0.5 * b[3 * H:4 * H]], 0)
    return Wp, bp  # [540, 180], [540]


def kernel(x, W_ih_enc, b_ih_enc, b_hh_enc, W_ih_dec, b_ih_dec, b_hh_dec,
           W_out, b_out):
    global LAST_RESULTS
    x = np.asarray(x)
    B = x.shape[0]
    assert B == B_FULL, f"kernel hardcoded for B={B_FULL}, got {B}"

    # x[b, c, s, t] with feature f = c*60+s; per core transpose to
    # [f, t, b] (column index t*BL + b) with a trailing ones row
    # providing the encoder bias via the augmented contraction dim.
    xf = x.reshape(B, H, T).astype(BF16)

    We, be = _prep_lstm_weights(W_ih_enc, b_ih_enc, b_hh_enc)
    wenc = np.concatenate([We.T, be[None, :]], 0).astype(BF16)  # [181, 540]

    Wd, bd = _prep_lstm_weights(W_ih_dec, b_ih_dec, b_hh_dec)
    # softmax rows sum to 1 -> bias folds into every column of Wdec
    wdec = (Wd.T + bd[None, :]).astype(BF16)  # [180, 540]

    # logits use h = h2/2 -> fold the 0.5 into W_out; W3[h, t, j]
    W3 = (0.5 * np.asarray(W_out, np.float32)).reshape(40, T, H)
    W3 = np.ascontiguousarray(W3.transpose(2, 1, 0))  # [180, 240, 40]
    w3a = np.ascontiguousarray(W3[0:128]).reshape(128, T * 40).astype(BF16)
    w3b = np.ascontiguousarray(W3[128:180]).reshape(52, T * 40).astype(BF16)

    bout = np.asarray(b_out, np.float32).reshape(40, 1)
    ident = np.eye(64, dtype=np.float32)

    nc = _get_program()
    in_maps = []
    for c in range(NCORES):
        xc = xf[c * BL:(c + 1) * BL]                       # [256, 180, 240]
        xtc = np.ascontiguousarray(xc.transpose(1, 2, 0))  # [180, 240, 256]
        xaug = np.empty((181, T * BL), BF16)
        xaug[:180] = xtc.reshape(H, T * BL)
        xaug[180] = 1.0
        in_maps.append({
            "xaug": xaug,
            "wenc": wenc,
            "wdec": wdec,
            "w3a": w3a,
            "w3b": w3b,
            "bout": bout,
            "ident": ident,
        })
    trace = bool(int(os.environ.get("KERNEL_TRACE", "0")))
    res = bass_utils.run_bass_kernel_spmd(
        nc, in_maps, core_ids=list(range(NCORES)), trace=trace)
    LAST_RESULTS = res
    out = np.concatenate([r["out"] for r in res.results], 0)  # [B, 40]
    return out.reshape(B, 4, 10).astype(np.float32)
